# revision 45
# baseline (speedup 1.0000x reference)
"""Trainium2 Bass kernel for nn_Attention_4045859193206 (Swin-style window
attention with relative position bias + key masking).

Contract: kernel(**inputs) takes FULL inputs (B=128 windows), shards the batch
across 8 NeuronCores (16 windows each), runs one SPMD Bass kernel, returns the
FULL (128, 196, 512) float32 output.

Self-contained: hardcodes all shapes; no sibling imports.

Shipping design (_VARIANT="v4"; _build_nc_v4 + _host_prep_v4, per core
W=16 windows):
  - Key compaction (host): the mask keeps ~99 of 196 keys per window (max
    113 for these inputs), so keys are compacted per window to KC=128 padded
    slots -> the whole attention phase runs in ONE k-chunk of 128 partitions
    (pad slots are killed by a -1e9 exp bias). Falls back to the 2-chunk
    _build_nc_v2 path if any window exceeded KC live keys.
  - Host prep also: pre-transposes+casts x to fp16 token streams (xt for
    queries, xkt for compacted keys; no on-device cast/transpose), folds the
    1/sqrt(d) scale into Wq, and precomputes exp(rpe bias) per window in
    compacted-key order ([kslot, h, q] fp16, streamed from DRAM per window).
  - QKV: Q^T/K^T in transposed form [o,tok] fp16 (PSUM->SBUF move adds the
    bias via per-partition tensor_scalar); V natural [kslot, o].
  - S = K^T-lhsT matmuls, 4 heads row-packed via tile_position(32i,0), one
    PSUM bank per head (two tile_position row streams into one bank hang
    real HW), pair/head-granular bank pools (bufs 2/4) for cross-head-group
    pipelining.
  - softmax: P = exp(S + mask_bias - 4) on ScalarE, then P *= exp(bias) on
    DVE (exp(S+B) = exp(S)*exp(B): turns the 2D bias add into an
    elementwise multiply; the -4 shift cancels in normalization).
  - O^T = V-lhsT matmuls col-packed over 4 heads (single k pass); Z via
    ones-matmul col-packed (replicated 32x so the reciprocal is
    partition-aligned). normalize O^T with reciprocal + tensor_mul.
  - proj in natural layout (lhsT = O^T chunks), bias added in the final
    PSUM->SBUF pass, fp16 store (host casts back to fp32).
"""

import contextlib
import numpy as np

import concourse.bass as bass
import concourse.mybir as mybir
import concourse.tile as tile
from concourse.bacc import Bacc

# ---------------------------------------------------------------- constants
B, N, DIM, H = 128, 196, 512, 16
HD = DIM // H                     # 32
RPE = 729                         # (2*14-1)^2
NCORES = 8
W = B // NCORES                   # 16 windows per core
NKC = 98                          # k-chunk (2 chunks of 98 = 196)
GW = 4                            # windows per qkv group (4*196=784 tokens)
F16 = mybir.dt.float16
F32 = mybir.dt.float32
I16 = mybir.dt.int16
EXP_SHIFT = -4.0                  # exp(s-4): fp16 headroom; cancels in softmax
MASK_NEG = -1e9
_GQ = 98                          # (c,q) positions per gather chunk
_NGATHER = 4                      # 4 chunks of 98 positions = 392


def _build_nc(n_w=W, ablate=frozenset(), variant="base", reps=1, pools=None):
    """Build the per-core Bass program for n_w windows.
    ablate: subset of {'z','bias','qk','pv'} - drop those matmuls (timing expts).
    variant: 'base' or 'bundle2' (2-head bias bundling + s_ps 2 banks x 2 bufs).
    reps: repeat the main loop in-kernel (slope timing; output identical)."""
    assert n_w % GW == 0
    ngrp = n_w // GW
    pl = {"xt": 2 if variant != "tune2" else 3,
          "qk": 2 if variant != "tune2" else 3,
          "v": 2,
          "p": 4 if variant in ("tune1", "tune2", "v2x", "v2x2", "v2x4") else 3,
          "e": 3, "o": 3 if variant == "tune2" else 2, "y": 3,
          "rz": 4 if variant == "tune2" else 3, "ps_a": 4}
    pl.update(pools or {})
    nc = Bacc("TRN2", target_bir_lowering=False)

    x_d = nc.dram_tensor("x", (n_w, N, DIM), F32, kind="ExternalInput")
    wqk_d = nc.dram_tensor("wqk", (4, 128, 2 * DIM), F16, kind="ExternalInput")
    wv_d = nc.dram_tensor("wv", (4, 128, DIM), F16, kind="ExternalInput")
    wp_d = nc.dram_tensor("wp", (4, 128, DIM), F16, kind="ExternalInput")
    bqk_d = nc.dram_tensor("bqk", (128, 8), F32, kind="ExternalInput")
    bv_d = nc.dram_tensor("bv", (DIM,), F32, kind="ExternalInput")
    bp_d = nc.dram_tensor("bp", (DIM,), F32, kind="ExternalInput")
    tab_d = nc.dram_tensor("tab", (RPE, 128), F16, kind="ExternalInput")
    idx_d = nc.dram_tensor("idx", (128, _GQ * 8 * _NGATHER), I16,
                           kind="ExternalInput")
    mb_d = nc.dram_tensor("mb", (NKC, n_w * 2), F32, kind="ExternalInput")
    ident_d = nc.dram_tensor("ident", (128, 128), F16, kind="ExternalInput")
    if variant in ("v2x", "v2x2", "v2x4"):
        expbt_d = nc.dram_tensor("expbt", (128, 2 * H * N), F16,
                                 kind="ExternalInput")
    out_d = nc.dram_tensor("out", (n_w, N, DIM), F32, kind="ExternalOutput")

    x16_d = nc.dram_tensor("x16", (n_w * N, DIM), F16)

    with tile.TileContext(nc) as tc, contextlib.ExitStack() as ctx:
        const = ctx.enter_context(tc.tile_pool(name="const", bufs=1))
        gpool = ctx.enter_context(tc.tile_pool(name="gather", bufs=2))
        xt_pool = ctx.enter_context(tc.tile_pool(name="xt", bufs=pl["xt"]))
        qk_pool = ctx.enter_context(tc.tile_pool(name="qk", bufs=pl["qk"]))
        v_pool = ctx.enter_context(tc.tile_pool(name="v", bufs=pl["v"]))
        p_pool = ctx.enter_context(tc.tile_pool(name="p", bufs=pl["p"]))
        if variant in ("v2x", "v2x2", "v2x4"):
            e_pool = ctx.enter_context(tc.tile_pool(name="e", bufs=pl["e"]))
        o_pool = ctx.enter_context(tc.tile_pool(name="o", bufs=pl["o"]))
        y_pool = ctx.enter_context(tc.tile_pool(name="y", bufs=pl["y"]))
        rz_pool = ctx.enter_context(tc.tile_pool(name="rz", bufs=pl["rz"]))
        if variant == "v2x4":
            ps_sc = [ctx.enter_context(tc.tile_pool(name=f"ps_s{j}", bufs=1,
                                                    space="PSUM"))
                     for j in range(2)]
        ps_s = ctx.enter_context(tc.tile_pool(
            name="ps_s", bufs=(2 if variant in ("bundle2", "v2x2") else 1),
            space="PSUM"))
        ps_a = ctx.enter_context(tc.tile_pool(name="ps_a", bufs=pl["ps_a"],
                                              space="PSUM"))

        # ---------------- constants ----------------
        wqk_sb = const.tile([128, 4, 2 * DIM], F16)   # [c128, ci, o]  (q|k)
        wv_sb = const.tile([128, 4, DIM], F16)
        wp_sb = const.tile([128, 4, DIM], F16)
        nc.sync.dma_start(out=wqk_sb, in_=wqk_d[:].rearrange("a b c -> b a c"))
        nc.sync.dma_start(out=wv_sb, in_=wv_d[:].rearrange("a b c -> b a c"))
        nc.sync.dma_start(out=wp_sb, in_=wp_d[:].rearrange("a b c -> b a c"))
        ident_sb = const.tile([128, 128], F16)
        nc.sync.dma_start(out=ident_sb, in_=ident_d[:])
        ones_sb = const.tile([NKC, HD], F16)
        nc.vector.memset(ones_sb, 1.0)
        mb_sb = const.tile([NKC, n_w * 2], F32)
        nc.sync.dma_start(out=mb_sb, in_=mb_d[:])
        bqk_sb = const.tile([128, 8], F32)            # per-partition qk bias
        nc.sync.dma_start(out=bqk_sb, in_=bqk_d[:])
        bv_bc = const.tile([128, DIM], F32)           # broadcast rows
        nc.sync.dma_start(
            out=bv_bc, in_=bass.AP(tensor=bv_d[:].tensor, offset=0,
                                   ap=[[0, 128], [1, DIM]]))
        bp_bc = const.tile([128, DIM], F32)
        nc.sync.dma_start(
            out=bp_bc, in_=bass.AP(tensor=bp_d[:].tensor, offset=0,
                                   ap=[[0, 128], [1, DIM]]))
        idx_sb = const.tile([128, _GQ * 8 * _NGATHER], I16)
        nc.sync.dma_start(out=idx_sb, in_=idx_d[:])
        if variant in ("v2x", "v2x2", "v2x4"):
            expbt_sb = const.tile([128, 2, H, N], F16)
            nc.sync.dma_start(
                out=expbt_sb.rearrange("p c h q -> p (c h q)"),
                in_=expbt_d[:])

        # gathered rpe bias: bias_sb[p, cq, h] = tab[idx[cq*128+p], h]
        bias_sb = const.tile([128, 2 * N, H], F16)
        for g in range(_NGATHER):
            g_sb = gpool.tile([128, _GQ, 128], F16, tag="gather")
            n_idx = _GQ * 128
            nc.gpsimd.dma_gather(
                out_ap=g_sb[:],
                in_ap=tab_d[:],
                idxs_ap=idx_sb[:, g * _GQ * 8:(g + 1) * _GQ * 8],
                num_idxs=n_idx,
                num_idxs_reg=n_idx,
                elem_size=128,
                single_packet=False,
            )
            nc.vector.tensor_copy(
                out=bias_sb[:, g * _GQ:(g + 1) * _GQ, :],
                in_=g_sb[:, :, 0:H],
            )

        # ---------------- main loop over 4-window groups ----------------
        for g in [gg for _ in range(reps) for gg in range(ngrp)]:
            tok0 = g * GW * N
            if "xcast" not in ablate:
                nc.gpsimd.dma_start(
                    out=x16_d[tok0:tok0 + GW * N, :],
                    in_=x_d[:].rearrange("w n c -> (w n) c")[tok0:tok0 + GW * N, :],
                )
            xt = xt_pool.tile([128, 4, GW * N], F16, tag="xt")
            for ci in range(4):
                nc.sync.dma_start_transpose(
                    out=xt[:, ci, :],
                    in_=x16_d[tok0:tok0 + GW * N, ci * 128:(ci + 1) * 128],
                )

            # Q^T / K^T  [o-chunk 128, tok] fp16
            qk_sb = qk_pool.tile([128, 8, GW * N], F16, tag="qk")
            for oc in range(8):
                for half in range(2):
                    mm_ps = ps_a.tile([128, 512], F32, tag="ps_a")
                    for ci in range(4):
                        nc.tensor.matmul(
                            mm_ps[:, 0:392],
                            lhsT=wqk_sb[:, ci, oc * 128:(oc + 1) * 128],
                            rhs=xt[:, ci, half * 392:(half + 1) * 392],
                            start=(ci == 0), stop=(ci == 3),
                        )
                    nc.any.tensor_scalar_add(
                        out=qk_sb[:, oc, half * 392:(half + 1) * 392],
                        in0=mm_ps[:, 0:392],
                        scalar1=bqk_sb[:, oc:oc + 1],
                    )

            # V natural  [tok-chunk 98, 512] fp16
            v_sb = v_pool.tile([NKC, GW, 2, DIM], F16, tag="v")
            for wi in range(GW):
                for tcn in range(2):
                    vv_ps = ps_a.tile([128, 512], F32, tag="ps_a")
                    for ci in range(4):
                        nc.tensor.matmul(
                            vv_ps[0:NKC, :],
                            lhsT=xt[:, ci,
                                    wi * N + tcn * NKC:wi * N + (tcn + 1) * NKC],
                            rhs=wv_sb[:, ci, :],
                            start=(ci == 0), stop=(ci == 3),
                        )
                    nc.vector.tensor_add(
                        out=v_sb[:, wi, tcn, :],
                        in0=vv_ps[0:NKC, :],
                        in1=bv_bc[0:NKC, :],
                    )

            # ---------------- attention per window ----------------
            for wi in range(GW):
                w_abs = g * GW + wi
                oT = o_pool.tile([128, 4, N], F16, tag="oT")
                for hg in range(4):
                    p_sb = p_pool.tile([NKC, 2, 4, N], F16, tag="p")
                    if variant not in ("bundle2", "v2x2", "v2x4"):
                        s_ps = ps_s.tile([128, 4, 512], F32, tag="s")
                    for c in range(2):
                        if variant == "v2x4":
                            s_ps = ps_sc[c].tile([128, 2, 512], F32, tag="s")
                        elif variant in ("bundle2", "v2x2"):
                            s_ps = ps_s.tile([128, 2, 512], F32, tag="s")
                        if "qk" in ablate and "bias" in ablate:
                            nc.tensor.matmul(
                                s_ps[0:NKC, 0, 0:32],
                                lhsT=ident_sb[0:NKC, 0:NKC],
                                rhs=bias_sb[0:NKC, 0:2, 0:16
                                            ].rearrange("p q h -> p (q h)"),
                                start=True, stop=True,
                            )
                        for i in range(4):               # head = 4*hg + i
                            if "qk" in ablate:
                                break
                            if variant in ("bundle2", "v2x2", "v2x4"):
                                s_out = s_ps[0:NKC, i // 2,
                                             (i % 2) * 196:(i % 2) * 196 + 196]
                                st = (i % 2 == 0) or variant in ("v2x2",
                                                                 "v2x4")
                            else:
                                s_out = s_ps[0:NKC, i, c * 196:c * 196 + 196]
                                st = True
                            if variant == "tune2":
                                nc.tensor.matmul(
                                    s_ps[0:NKC, i, c * 196:c * 196 + 196],
                                    lhsT=ident_sb[0:NKC, 0:NKC],
                                    rhs=bias_sb[0:NKC, c * N:(c + 1) * N,
                                                4 * hg + i],
                                    start=True, stop=False,
                                )
                            nc.tensor.matmul(
                                s_out,
                                lhsT=qk_sb[32 * i:32 * (i + 1), 4 + hg,
                                           wi * N + c * NKC:
                                           wi * N + (c + 1) * NKC],
                                rhs=qk_sb[32 * i:32 * (i + 1), hg,
                                          wi * N:(wi + 1) * N],
                                start=(st and variant != "tune2"),
                                stop=(variant in ("tune2", "v2x", "v2x2",
                                                  "v2x4")),
                                tile_position=(32 * i, 0),
                            )
                        if ("bias" not in ablate and variant == "bundle2"
                                and variant != "v2x2"):
                            for pr in range(2):          # head pair
                                nc.tensor.matmul(
                                    s_ps[0:NKC, pr, 0:392],
                                    lhsT=ident_sb[0:NKC, 0:NKC],
                                    rhs=bias_sb[0:NKC, c * N:(c + 1) * N,
                                                4 * hg + 2 * pr:
                                                4 * hg + 2 * pr + 2
                                                ].rearrange("p q h -> p h q"),
                                    start=("qk" in ablate), stop=True,
                                )
                        elif ("bias" not in ablate
                              and variant not in ("tune2", "v2x", "v2x2",
                                                  "v2x4")):
                            for i in range(4):           # rpe bias, K=98 each
                                h = 4 * hg + i
                                nc.tensor.matmul(
                                    s_ps[0:NKC, i, c * 196:c * 196 + 196],
                                    lhsT=ident_sb[0:NKC, 0:NKC],
                                    rhs=bias_sb[0:NKC, c * N:(c + 1) * N, h],
                                    start=("qk" in ablate), stop=True,
                                )
                        if variant == "batch":
                            continue                     # exps after all MMs
                        if variant in ("bundle2", "v2x2", "v2x4"):
                            exp_in = s_ps[0:NKC, :, 0:392]
                        else:
                            exp_in = s_ps[0:NKC, :, c * 196:c * 196 + 196]
                        if variant in ("v2x", "v2x2", "v2x4"):
                            e_sb = e_pool.tile([NKC, 4, N], F16, tag="e")
                            nc.scalar.activation(
                                out=e_sb[:],
                                in_=exp_in,
                                func=mybir.ActivationFunctionType.Exp,
                                bias=mb_sb[:, 2 * w_abs + c:
                                           2 * w_abs + c + 1],
                                scale=1.0,
                            )
                            nc.vector.tensor_mul(
                                out=p_sb[:, c, :, :],
                                in0=e_sb[:],
                                in1=expbt_sb[0:NKC, c, 4 * hg:4 * hg + 4, :],
                            )
                            continue
                        nc.scalar.activation(
                            out=p_sb[:, c, :, :],
                            in_=exp_in,
                            func=mybir.ActivationFunctionType.Exp,
                            bias=mb_sb[:, 2 * w_abs + c:2 * w_abs + c + 1],
                            scale=1.0,
                        )
                    if variant == "batch":
                        for c in range(2):
                            nc.scalar.activation(
                                out=p_sb[:, c, :, :],
                                in_=s_ps[0:NKC, :, c * 196:c * 196 + 196],
                                func=mybir.ActivationFunctionType.Exp,
                                bias=mb_sb[:, 2 * w_abs + c:
                                           2 * w_abs + c + 1],
                                scale=1.0,
                            )
                    # PV + Z, col-packed over the 4 heads
                    o_ps = ps_a.tile([128, 512], F32, tag="ps_a")
                    z_ps = ps_a.tile([128, 512], F32, tag="ps_a")
                    for i in range(4):
                        h = 4 * hg + i
                        if "pv" in ablate and i == 0:
                            nc.tensor.matmul(
                                o_ps[0:32, 0:16],
                                lhsT=v_sb[:, wi, 0, 0:32],
                                rhs=p_sb[:, 0, 0, 0:16],
                                start=True, stop=True,
                            )
                        if "pv" not in ablate:
                            for c in range(2):
                                nc.tensor.matmul(
                                    o_ps[32 * i:32 * (i + 1), 0:N],
                                    lhsT=v_sb[:, wi, c, 32 * h:32 * (h + 1)],
                                    rhs=p_sb[:, c, i, :],
                                    start=(c == 0), stop=(c == 1),
                                    tile_position=(0, 32 * i),
                                )
                        if "z" in ablate and i == 0:
                            nc.tensor.matmul(
                                z_ps[0:32, 0:16],
                                lhsT=ones_sb[:, 0:32],
                                rhs=p_sb[:, 0, 0, 0:16],
                                start=True, stop=True,
                            )
                        if "z" not in ablate:
                            for c in range(2):
                                nc.tensor.matmul(
                                    z_ps[32 * i:32 * (i + 1), 0:N],
                                    lhsT=ones_sb[:],
                                    rhs=p_sb[:, c, i, :],
                                    start=(c == 0), stop=(c == 1),
                                    tile_position=(0, 32 * i),
                                )
                    rz = rz_pool.tile([128, N], F32, tag="rz")
                    if variant in ("tune1", "tune2"):
                        nc.vector.reciprocal(out=rz[:], in_=z_ps[:, 0:N])
                    else:
                        z_sb = rz_pool.tile([128, N], F32, tag="z")
                        nc.scalar.copy(out=z_sb[:], in_=z_ps[:, 0:N])
                        nc.vector.reciprocal_approx_fast(out=rz[:], in_=z_sb[:])
                    nc.vector.tensor_mul(
                        out=oT[:, hg, :], in0=o_ps[:, 0:N], in1=rz[:])

                # ---------------- proj ----------------
                for qc in range(2):
                    y_ps = ps_a.tile([128, 512], F32, tag="ps_a")
                    for hg in range(4):
                        nc.tensor.matmul(
                            y_ps[0:NKC, :],
                            lhsT=oT[:, hg, qc * NKC:(qc + 1) * NKC],
                            rhs=wp_sb[:, hg, :],
                            start=(hg == 0), stop=(hg == 3),
                        )
                    y_sb = y_pool.tile([NKC, DIM], F32, tag="y")
                    nc.vector.tensor_add(
                        out=y_sb[:], in0=y_ps[0:NKC, :], in1=bp_bc[0:NKC, :])
                    if "outdma" not in ablate:
                        nc.sync.dma_start(
                            out=out_d[w_abs, qc * NKC:(qc + 1) * NKC, :],
                            in_=y_sb[:],
                        )
    nc.compile()
    return nc


def _build_nc_v2(n_w=W, ablate=frozenset(), reps=1, s2=False, sgran="hg",
                 sbufs=None):
    """v2: RPE bias folded into softmax as exp(S)*exp(B); exp(B) is computed
    host-side (input prep, like the weight transposes) and uploaded in
    [k-part, chunk, head, q] layout, so the per-window identity-matmul bias
    replay (1024 matmuls, ~100k PE rows) and the on-device gather disappear.
    s_ps shrinks to 2 banks x 2 bufs for cross-head-group overlap."""
    assert n_w % GW == 0
    ngrp = n_w // GW
    nc = Bacc("TRN2", target_bir_lowering=False)

    x_d = nc.dram_tensor("x", (n_w, N, DIM), F32, kind="ExternalInput")
    wqk_d = nc.dram_tensor("wqk", (4, 128, 2 * DIM), F16, kind="ExternalInput")
    wv_d = nc.dram_tensor("wv", (4, 128, DIM), F16, kind="ExternalInput")
    wp_d = nc.dram_tensor("wp", (4, 128, DIM), F16, kind="ExternalInput")
    bqk_d = nc.dram_tensor("bqk", (128, 8), F32, kind="ExternalInput")
    bv_d = nc.dram_tensor("bv", (DIM,), F32, kind="ExternalInput")
    bp_d = nc.dram_tensor("bp", (DIM,), F32, kind="ExternalInput")
    expbt_d = nc.dram_tensor("expbt", (128, 2 * H * N), F16,
                             kind="ExternalInput")
    mb_d = nc.dram_tensor("mb", (NKC, n_w * 2), F32, kind="ExternalInput")
    out_d = nc.dram_tensor("out", (n_w, N, DIM), F32, kind="ExternalOutput")

    x16_d = nc.dram_tensor("x16", (n_w * N, DIM), F16)

    with tile.TileContext(nc) as tc, contextlib.ExitStack() as ctx:
        const = ctx.enter_context(tc.tile_pool(name="const", bufs=1))
        xt_pool = ctx.enter_context(tc.tile_pool(name="xt", bufs=2))
        qk_pool = ctx.enter_context(tc.tile_pool(name="qk", bufs=2))
        v_pool = ctx.enter_context(tc.tile_pool(name="v", bufs=2))
        p_pool = ctx.enter_context(tc.tile_pool(name="p", bufs=4))
        e_pool = ctx.enter_context(tc.tile_pool(name="e", bufs=3))
        o_pool = ctx.enter_context(tc.tile_pool(name="o", bufs=2))
        y_pool = ctx.enter_context(tc.tile_pool(name="y", bufs=3))
        rz_pool = ctx.enter_context(tc.tile_pool(name="rz", bufs=4))
        if sbufs is None:
            sbufs = {"hg": 2 if s2 else 1, "pair": 2, "head": 4}[sgran]
        ps_s = ctx.enter_context(tc.tile_pool(name="ps_s", bufs=sbufs,
                                              space="PSUM"))
        ps_a = ctx.enter_context(tc.tile_pool(name="ps_a", bufs=pl["ps_a"],
                                              space="PSUM"))

        # ---------------- constants ----------------
        wqk_sb = const.tile([128, 4, 2 * DIM], F16)   # [c128, ci, o]  (q|k)
        wv_sb = const.tile([128, 4, DIM], F16)
        wp_sb = const.tile([128, 4, DIM], F16)
        nc.sync.dma_start(out=wqk_sb, in_=wqk_d[:].rearrange("a b c -> b a c"))
        nc.sync.dma_start(out=wv_sb, in_=wv_d[:].rearrange("a b c -> b a c"))
        nc.sync.dma_start(out=wp_sb, in_=wp_d[:].rearrange("a b c -> b a c"))
        ones_sb = const.tile([NKC, HD], F16)
        nc.vector.memset(ones_sb, 1.0)
        mb_sb = const.tile([NKC, n_w * 2], F32)
        nc.sync.dma_start(out=mb_sb, in_=mb_d[:])
        bqk_sb = const.tile([128, 8], F32)            # per-partition qk bias
        nc.sync.dma_start(out=bqk_sb, in_=bqk_d[:])
        bv_bc = const.tile([128, DIM], F32)           # broadcast rows
        nc.sync.dma_start(
            out=bv_bc, in_=bass.AP(tensor=bv_d[:].tensor, offset=0,
                                   ap=[[0, 128], [1, DIM]]))
        bp_bc = const.tile([128, DIM], F32)
        nc.sync.dma_start(
            out=bp_bc, in_=bass.AP(tensor=bp_d[:].tensor, offset=0,
                                   ap=[[0, 128], [1, DIM]]))
        expbt_sb = const.tile([128, 2, H, N], F16)    # [k-part, c, h, q]
        if "ebdma" not in ablate:
            nc.sync.dma_start(
                out=expbt_sb.rearrange("p c h q -> p (c h q)"),
                in_=expbt_d[:])

        # ---------------- main loop over 4-window groups ----------------
        for g in [gg for _ in range(reps) for gg in range(ngrp)]:
            tok0 = g * GW * N
            if "xcast" not in ablate:
                nc.gpsimd.dma_start(
                    out=x16_d[tok0:tok0 + GW * N, :],
                    in_=x_d[:].rearrange("w n c -> (w n) c")[
                        tok0:tok0 + GW * N, :],
                )
            xt = xt_pool.tile([128, 4, GW * N], F16, tag="xt")
            for ci in range(4):
                nc.sync.dma_start_transpose(
                    out=xt[:, ci, :],
                    in_=x16_d[tok0:tok0 + GW * N, ci * 128:(ci + 1) * 128],
                )

            # Q^T / K^T  [o-chunk 128, tok] fp16
            qk_sb = qk_pool.tile([128, 8, GW * N], F16, tag="qk")
            for oc in range(8):
                for half in range(2):
                    mm_ps = ps_a.tile([128, 512], F32, tag="ps_a")
                    for ci in range(4):
                        nc.tensor.matmul(
                            mm_ps[:, 0:392],
                            lhsT=wqk_sb[:, ci, oc * 128:(oc + 1) * 128],
                            rhs=xt[:, ci, half * 392:(half + 1) * 392],
                            start=(ci == 0), stop=(ci == 3),
                        )
                    nc.any.tensor_scalar_add(
                        out=qk_sb[:, oc, half * 392:(half + 1) * 392],
                        in0=mm_ps[:, 0:392],
                        scalar1=bqk_sb[:, oc:oc + 1],
                    )

            # V natural  [tok-chunk 98, 512] fp16
            v_sb = v_pool.tile([NKC, GW, 2, DIM], F16, tag="v")
            for wi in range(GW):
                for tcn in range(2):
                    vv_ps = ps_a.tile([128, 512], F32, tag="ps_a")
                    for ci in range(4):
                        nc.tensor.matmul(
                            vv_ps[0:NKC, :],
                            lhsT=xt[:, ci,
                                    wi * N + tcn * NKC:wi * N + (tcn + 1) * NKC],
                            rhs=wv_sb[:, ci, :],
                            start=(ci == 0), stop=(ci == 3),
                        )
                    nc.vector.tensor_add(
                        out=v_sb[:, wi, tcn, :],
                        in0=vv_ps[0:NKC, :],
                        in1=bv_bc[0:NKC, :],
                    )

            # ---------------- attention per window ----------------
            for wi in range(GW):
                w_abs = g * GW + wi
                oT = o_pool.tile([128, 4, N], F16, tag="oT")
                for hg in range(4):
                    p_sb = p_pool.tile([NKC, 2, 4, N], F16, tag="p")
                    if sgran == "hg":
                        s_tiles = [ps_s.tile([128, 4, 512], F32, tag="s",
                                             name=f"sps_{hg}")]
                    elif sgran == "pair":
                        s_tiles = [ps_s.tile([128, 2, 512], F32, tag="s",
                                             name=f"sps_{hg}_{j}")
                                   for j in range(2)]
                    else:
                        s_tiles = [ps_s.tile([128, 1, 512], F32, tag="s",
                                             name=f"sps_{hg}_{j}")
                                   for j in range(4)]

                    def s_out(i, c):
                        if sgran == "hg":
                            return s_tiles[0][0:NKC, i,
                                              c * 196:c * 196 + 196]
                        if sgran == "pair":
                            return s_tiles[i // 2][0:NKC, i % 2,
                                                   c * 196:c * 196 + 196]
                        return s_tiles[i][0:NKC, 0, c * 196:c * 196 + 196]

                    for c in range(2):
                        if "qk" not in ablate:
                            for i in range(4):           # head = 4*hg + i
                                nc.tensor.matmul(
                                    s_out(i, c),
                                    lhsT=qk_sb[32 * i:32 * (i + 1), 4 + hg,
                                               wi * N + c * NKC:
                                               wi * N + (c + 1) * NKC],
                                    rhs=qk_sb[32 * i:32 * (i + 1), hg,
                                              wi * N:(wi + 1) * N],
                                    start=True, stop=True,
                                    tile_position=(32 * i, 0),
                                )
                        if sgran == "hg":
                            acts = [(s_tiles[0][0:NKC, :,
                                                c * 196:c * 196 + 196],
                                     slice(0, 4))]
                        elif sgran == "pair":
                            acts = [(s_tiles[j][0:NKC, :,
                                                c * 196:c * 196 + 196],
                                     slice(2 * j, 2 * j + 2))
                                    for j in range(2)]
                        else:
                            acts = [(s_tiles[i][0:NKC, 0,
                                                c * 196:c * 196 + 196],
                                     slice(i, i + 1))
                                    for i in range(4)]
                        e_sb = e_pool.tile([NKC, 4, N], F16, tag="e")
                        dst = p_sb if "bias" in ablate else e_sb
                        for in_ap, hs in acts:
                            out_ap = (p_sb[:, c, hs, :] if "bias" in ablate
                                      else e_sb[:, hs, :])
                            nc.scalar.activation(
                                out=out_ap,
                                in_=in_ap,
                                func=mybir.ActivationFunctionType.Exp,
                                bias=mb_sb[:, 2 * w_abs + c:2 * w_abs + c + 1],
                                scale=1.0,
                            )
                        if "bias" not in ablate:
                            nc.vector.tensor_mul(
                                out=p_sb[:, c, :, :],
                                in0=e_sb[:],
                                in1=expbt_sb[0:NKC, c, 4 * hg:4 * hg + 4, :],
                            )
                    # PV + Z, col-packed over the 4 heads
                    o_ps = ps_a.tile([128, 512], F32, tag="ps_a")
                    z_ps = ps_a.tile([128, 512], F32, tag="ps_a")
                    for i in range(4):
                        h = 4 * hg + i
                        if "pv" not in ablate:
                            for c in range(2):
                                nc.tensor.matmul(
                                    o_ps[32 * i:32 * (i + 1), 0:N],
                                    lhsT=v_sb[:, wi, c, 32 * h:32 * (h + 1)],
                                    rhs=p_sb[:, c, i, :],
                                    start=(c == 0), stop=(c == 1),
                                    tile_position=(0, 32 * i),
                                )
                        if "z" not in ablate:
                            for c in range(2):
                                nc.tensor.matmul(
                                    z_ps[32 * i:32 * (i + 1), 0:N],
                                    lhsT=ones_sb[:],
                                    rhs=p_sb[:, c, i, :],
                                    start=(c == 0), stop=(c == 1),
                                    tile_position=(0, 32 * i),
                                )
                    if fdiv:
                        nc.vector.tensor_tensor(
                            out=oT[:, hg, :], in0=o_ps[:, 0:N],
                            in1=z_ps[:, 0:N], op=mybir.AluOpType.divide)
                    else:
                        rz = rz_pool.tile([128, N], F32, tag="rz")
                        nc.vector.reciprocal(out=rz[:], in_=z_ps[:, 0:N])
                        (nc.any if anyeng else nc.vector).tensor_mul(
                            out=oT[:, hg, :], in0=o_ps[:, 0:N], in1=rz[:])

                # ---------------- proj ----------------
                for qc in range(2):
                    y_ps = ps_a.tile([128, 512], F32, tag="ps_a")
                    for hg in range(4):
                        nc.tensor.matmul(
                            y_ps[0:NKC, :],
                            lhsT=oT[:, hg, qc * NKC:(qc + 1) * NKC],
                            rhs=wp_sb[:, hg, :],
                            start=(hg == 0), stop=(hg == 3),
                        )
                    y_sb = y_pool.tile([NKC, DIM], F32, tag="y")
                    nc.vector.tensor_add(
                        out=y_sb[:], in0=y_ps[0:NKC, :], in1=bp_bc[0:NKC, :])
                    if "outdma" not in ablate:
                        nc.sync.dma_start(
                            out=out_d[w_abs, qc * NKC:(qc + 1) * NKC, :],
                            in_=y_sb[:],
                        )
    nc.compile()
    return nc


KC = 128                          # compacted key slots per window (v4)


def _build_nc_v4(n_w=W, reps=1, sbufs=None, ablate=frozenset(), out16=True,
                 pools=None, sgran="head", anyeng=False, interleave=False,
                 ebwpre=False, fdiv=False):
    """v4: host-compacted keys (mask==1 rows only, padded to KC=128).
    One k-chunk per window: S/exp/PV/Z halve vs v2. Q/K token streams are
    host-pretransposed fp16 uploads (no on-device cast/transpose). Per-window
    exp(rpe bias) for the compacted key order streams from DRAM."""
    assert n_w % GW == 0
    ngrp = n_w // GW
    pl = {"xt": 2, "xkt": 2, "q": 2, "k": 2, "v": 2, "ebw": 3, "p": 4,
          "e": 3, "o": 2, "y": 3, "rz": 4, "ps_a": 4}
    pl.update(pools or {})
    if sbufs is None:
        sbufs = {"head": 4, "pair": 2}[sgran]
    OD = F16 if out16 else F32
    nc = Bacc("TRN2", target_bir_lowering=False)

    xt_d = nc.dram_tensor("xt", (4, 128, n_w * N), F16, kind="ExternalInput")
    xkt_d = nc.dram_tensor("xkt", (4, 128, n_w * KC), F16,
                           kind="ExternalInput")
    wqk_d = nc.dram_tensor("wqk", (4, 128, 2 * DIM), F16, kind="ExternalInput")
    wv_d = nc.dram_tensor("wv", (4, 128, DIM), F16, kind="ExternalInput")
    wp_d = nc.dram_tensor("wp", (4, 128, DIM), F16, kind="ExternalInput")
    bqk_d = nc.dram_tensor("bqk", (128, 8), F32, kind="ExternalInput")
    bv_d = nc.dram_tensor("bv", (DIM,), F32, kind="ExternalInput")
    bp_d = nc.dram_tensor("bp", (DIM,), F32, kind="ExternalInput")
    ebw_d = nc.dram_tensor("ebw", (n_w, 128, H * N), F16,
                           kind="ExternalInput")
    mbk_d = nc.dram_tensor("mbk", (128, n_w), F32, kind="ExternalInput")
    out_d = nc.dram_tensor("out", (n_w, N, DIM), OD, kind="ExternalOutput")

    with tile.TileContext(nc) as tc, contextlib.ExitStack() as ctx:
        const = ctx.enter_context(tc.tile_pool(name="const", bufs=1))
        xt_pool = ctx.enter_context(tc.tile_pool(name="xt", bufs=pl["xt"]))
        xkt_pool = ctx.enter_context(tc.tile_pool(name="xkt",
                                                  bufs=pl["xkt"]))
        q_pool = ctx.enter_context(tc.tile_pool(name="q", bufs=pl["q"]))
        k_pool = ctx.enter_context(tc.tile_pool(name="k", bufs=pl["k"]))
        v_pool = ctx.enter_context(tc.tile_pool(name="v", bufs=pl["v"]))
        ebw_pool = ctx.enter_context(tc.tile_pool(name="ebw",
                                                  bufs=pl["ebw"]))
        p_pool = ctx.enter_context(tc.tile_pool(name="p", bufs=pl["p"]))
        e_pool = ctx.enter_context(tc.tile_pool(name="e", bufs=pl["e"]))
        o_pool = ctx.enter_context(tc.tile_pool(name="o", bufs=pl["o"]))
        y_pool = ctx.enter_context(tc.tile_pool(name="y", bufs=pl["y"]))
        rz_pool = ctx.enter_context(tc.tile_pool(name="rz", bufs=pl["rz"]))
        ps_s = ctx.enter_context(tc.tile_pool(name="ps_s", bufs=sbufs,
                                              space="PSUM"))
        ps_a = ctx.enter_context(tc.tile_pool(name="ps_a", bufs=pl["ps_a"],
                                              space="PSUM"))

        # ---------------- constants ----------------
        wqk_sb = const.tile([128, 4, 2 * DIM], F16)   # [c128, ci, o]  (q|k)
        wv_sb = const.tile([128, 4, DIM], F16)
        wp_sb = const.tile([128, 4, DIM], F16)
        nc.sync.dma_start(out=wqk_sb, in_=wqk_d[:].rearrange("a b c -> b a c"))
        nc.sync.dma_start(out=wv_sb, in_=wv_d[:].rearrange("a b c -> b a c"))
        nc.sync.dma_start(out=wp_sb, in_=wp_d[:].rearrange("a b c -> b a c"))
        ones_sb = const.tile([128, HD], F16)
        nc.vector.memset(ones_sb, 1.0)
        mbk_sb = const.tile([128, n_w], F32)
        nc.sync.dma_start(out=mbk_sb, in_=mbk_d[:])
        bqk_sb = const.tile([128, 8], F32)            # per-partition q|k bias
        nc.sync.dma_start(out=bqk_sb, in_=bqk_d[:])
        bv_bc = const.tile([128, DIM], F32)           # broadcast rows
        nc.sync.dma_start(
            out=bv_bc, in_=bass.AP(tensor=bv_d[:].tensor, offset=0,
                                   ap=[[0, 128], [1, DIM]]))
        bp_bc = const.tile([128, DIM], F32)
        nc.sync.dma_start(
            out=bp_bc, in_=bass.AP(tensor=bp_d[:].tensor, offset=0,
                                   ap=[[0, 128], [1, DIM]]))

        # ---------------- qkv emission (per 4-window group) ----------------
        def emit_group_qkv(g):
            """Allocate tiles + DMAs for group g; return (tiles, units) where
            units are 16 deferred closures, each one qkv matmul chain. With
            interleave=True the units are emitted inside the PREVIOUS group's
            attention to fill PE gaps while Act/DVE work."""
            xt = xt_pool.tile([128, 4, GW * N], F16, tag="xt", name=f"xt{g}")
            xkt = xkt_pool.tile([128, 4, GW * KC], F16, tag="xkt",
                                name=f"xkt{g}")
            nc.sync.dma_start(
                out=xt,
                in_=xt_d[:, :, g * GW * N:(g + 1) * GW * N
                         ].rearrange("a b c -> b a c"))
            nc.sync.dma_start(
                out=xkt,
                in_=xkt_d[:, :, g * GW * KC:(g + 1) * GW * KC
                          ].rearrange("a b c -> b a c"))
            q_sb = q_pool.tile([128, 4, GW * N], F16, tag="q", name=f"q{g}")
            k_sb = k_pool.tile([128, 4, GW * KC], F16, tag="k", name=f"k{g}")
            v_sb = v_pool.tile([128, GW, DIM], F16, tag="v", name=f"v{g}")

            def q_unit(oc, half):
                def emit():
                    mm_ps = ps_a.tile([128, 512], F32, tag="ps_a")
                    for ci in range(4):
                        nc.tensor.matmul(
                            mm_ps[:, 0:392],
                            lhsT=wqk_sb[:, ci, oc * 128:(oc + 1) * 128],
                            rhs=xt[:, ci, half * 392:(half + 1) * 392],
                            start=(ci == 0), stop=(ci == 3),
                        )
                    nc.any.tensor_scalar_add(
                        out=q_sb[:, oc, half * 392:(half + 1) * 392],
                        in0=mm_ps[:, 0:392],
                        scalar1=bqk_sb[:, oc:oc + 1],
                    )
                return emit

            def k_unit(oc):
                def emit():
                    mm_ps = ps_a.tile([128, 512], F32, tag="ps_a")
                    for ci in range(4):
                        nc.tensor.matmul(
                            mm_ps[:, 0:512],
                            lhsT=wqk_sb[:, ci,
                                        512 + oc * 128:512 + (oc + 1) * 128],
                            rhs=xkt[:, ci, :],
                            start=(ci == 0), stop=(ci == 3),
                        )
                    nc.any.tensor_scalar_add(
                        out=k_sb[:, oc, :],
                        in0=mm_ps[:, 0:512],
                        scalar1=bqk_sb[:, 4 + oc:4 + oc + 1],
                    )
                return emit

            def v_unit(wi):
                def emit():
                    vv_ps = ps_a.tile([128, 512], F32, tag="ps_a")
                    for ci in range(4):
                        nc.tensor.matmul(
                            vv_ps[:, :],
                            lhsT=xkt[:, ci, wi * KC:(wi + 1) * KC],
                            rhs=wv_sb[:, ci, :],
                            start=(ci == 0), stop=(ci == 3),
                        )
                    (nc.any if anyeng else nc.vector).tensor_add(
                        out=v_sb[:, wi, :], in0=vv_ps[:, :], in1=bv_bc[:, :])
                return emit

            units = ([q_unit(oc, half) for oc in range(4) for half in range(2)]
                     + [k_unit(oc) for oc in range(4)]
                     + [v_unit(wi) for wi in range(GW)])
            return (q_sb, k_sb, v_sb), units

        # ---------------- main loop over 4-window groups ----------------
        glist = [gg for _ in range(reps) for gg in range(ngrp)]
        # flat window list for cross-group ebw prefetch
        wflat = [(gi, gg * GW + wi) for gi, gg in enumerate(glist)
                 for wi in range(GW)]
        ebw_tiles = {}

        def load_ebw(j):
            if j >= len(wflat) or j in ebw_tiles:
                return
            w_abs = wflat[j][1]
            t = ebw_pool.tile([128, H, N], F16, tag="ebw", name=f"ebw{j}")
            nc.sync.dma_start(
                out=t.rearrange("p h q -> p (h q)"),
                in_=ebw_d[w_abs, :, :])
            ebw_tiles[j] = t

        tiles_cur, units_cur = emit_group_qkv(glist[0])
        for u in units_cur:
            u()
        for gidx, g in enumerate(glist):
            q_sb, k_sb, v_sb = tiles_cur
            if gidx + 1 < len(glist):
                tiles_nxt, units_nxt = emit_group_qkv(glist[gidx + 1])
            else:
                tiles_nxt, units_nxt = None, []
            ui = 0   # interleave: emitted in attention gaps; else all at tail

            # ---------------- attention per window ----------------
            for wi in range(GW):
                w_abs = g * GW + wi
                j = gidx * GW + wi
                if ebwpre:
                    load_ebw(j)          # no-op if prefetched
                    load_ebw(j + 1)      # prefetch next window's bias
                    ebw_sb = ebw_tiles.pop(j)
                else:
                    load_ebw(j)
                    ebw_sb = ebw_tiles.pop(j)
                oT = o_pool.tile([128, 4, N], F16, tag="oT")
                for hg in range(4):
                    p_sb = p_pool.tile([128, 4, N], F16, tag="p")
                    e_sb = e_pool.tile([128, 4, N], F16, tag="e")
                    if sgran == "head":
                        s_tiles = [ps_s.tile([128, 1, 512], F32, tag="s",
                                             name=f"sps_{hg}_{j}")
                                   for j in range(4)]

                        def s_out(i):
                            return s_tiles[i][:, 0, 0:196]

                        acts = [(s_tiles[i][:, 0, 0:196], slice(i, i + 1))
                                for i in range(4)]
                    else:
                        s_tiles = [ps_s.tile([128, 2, 512], F32, tag="s",
                                             name=f"sps_{hg}_{j}")
                                   for j in range(2)]

                        def s_out(i):
                            return s_tiles[i // 2][:, i % 2, 0:196]

                        acts = [(s_tiles[j][:, :, 0:196],
                                 slice(2 * j, 2 * j + 2)) for j in range(2)]
                    for i in range(4):                   # head = 4*hg + i
                        if "qk" in ablate:
                            break
                        nc.tensor.matmul(
                            s_out(i),
                            lhsT=k_sb[32 * i:32 * (i + 1), hg,
                                      wi * KC:(wi + 1) * KC],
                            rhs=q_sb[32 * i:32 * (i + 1), hg,
                                     wi * N:(wi + 1) * N],
                            start=True, stop=True,
                            tile_position=(32 * i, 0),
                        )
                    if interleave and ui < len(units_nxt):
                        units_nxt[ui]()      # fill PE gap with next-group qkv
                        ui += 1
                    for in_ap, hs in acts:
                        nc.scalar.activation(
                            out=e_sb[:, hs, :],
                            in_=in_ap,
                            func=mybir.ActivationFunctionType.Exp,
                            bias=mbk_sb[:, w_abs:w_abs + 1],
                            scale=1.0,
                        )
                    if "bias" in ablate:
                        p_sb = e_sb
                    else:
                        (nc.any if anyeng else nc.vector).tensor_mul(
                            out=p_sb[:],
                            in0=e_sb[:],
                            in1=ebw_sb[:, 4 * hg:4 * hg + 4, :],
                        )
                    # PV + Z, col-packed over the 4 heads, single k pass
                    o_ps = ps_a.tile([128, 512], F32, tag="ps_a")
                    z_ps = ps_a.tile([128, 512], F32, tag="ps_a")
                    for i in range(4):
                        h = 4 * hg + i
                        if "pv" not in ablate:
                            nc.tensor.matmul(
                                o_ps[32 * i:32 * (i + 1), 0:N],
                                lhsT=v_sb[:, wi, 32 * h:32 * (h + 1)],
                                rhs=p_sb[:, i, :],
                                start=True, stop=True,
                                tile_position=(0, 32 * i),
                            )
                        if "z" not in ablate:
                            nc.tensor.matmul(
                                z_ps[32 * i:32 * (i + 1), 0:N],
                                lhsT=ones_sb[:],
                                rhs=p_sb[:, i, :],
                                start=True, stop=True,
                                tile_position=(0, 32 * i),
                            )
                    if fdiv:
                        nc.vector.tensor_tensor(
                            out=oT[:, hg, :], in0=o_ps[:, 0:N],
                            in1=z_ps[:, 0:N], op=mybir.AluOpType.divide)
                    else:
                        rz = rz_pool.tile([128, N], F32, tag="rz")
                        nc.vector.reciprocal(out=rz[:], in_=z_ps[:, 0:N])
                        (nc.any if anyeng else nc.vector).tensor_mul(
                            out=oT[:, hg, :], in0=o_ps[:, 0:N], in1=rz[:])

                # ---------------- proj ----------------
                for qc in range(2):
                    y_ps = ps_a.tile([128, 512], F32, tag="ps_a")
                    for hg in range(4):
                        nc.tensor.matmul(
                            y_ps[0:NKC, :],
                            lhsT=oT[:, hg, qc * NKC:(qc + 1) * NKC],
                            rhs=wp_sb[:, hg, :],
                            start=(hg == 0), stop=(hg == 3),
                        )
                    y_sb = y_pool.tile([NKC, DIM], OD, tag="y")
                    (nc.any if anyeng else nc.vector).tensor_add(
                        out=y_sb[:], in0=y_ps[0:NKC, :], in1=bp_bc[0:NKC, :])
                    nc.sync.dma_start(
                        out=out_d[w_abs, qc * NKC:(qc + 1) * NKC, :],
                        in_=y_sb[:],
                    )
            while ui < len(units_nxt):
                units_nxt[ui]()
                ui += 1
            tiles_cur = tiles_nxt
    nc.compile()
    return nc


def _host_prep_v4(x, rpe_index, mask, qkv_w, qkv_b, proj_w, proj_b, rpe_table,
                  n_w=W, n_cores=NCORES):
    """v4 host prep: per-window key compaction + pretransposed fp16 streams."""
    x = np.asarray(x, dtype=np.float32)
    rpe_index = np.asarray(rpe_index).astype(np.int64)
    mask = np.asarray(mask).astype(np.int32)
    qkv_w = np.asarray(qkv_w, dtype=np.float32)
    qkv_b = np.asarray(qkv_b, dtype=np.float32)
    proj_w = np.asarray(proj_w, dtype=np.float32)
    proj_b = np.asarray(proj_b, dtype=np.float32)
    rpe_table = np.asarray(rpe_table, dtype=np.float32)

    scale = HD ** -0.5
    wq = qkv_w[0:DIM] * scale
    wk = qkv_w[DIM:2 * DIM]
    wv = qkv_w[2 * DIM:3 * DIM]
    wqk_t = np.ascontiguousarray(
        np.concatenate([wq, wk], axis=0).T.astype(np.float16)
        .reshape(4, 128, 2 * DIM))
    wv_t = np.ascontiguousarray(wv.T.astype(np.float16).reshape(4, 128, DIM))
    wp_t = np.ascontiguousarray(
        proj_w.T.astype(np.float16).reshape(4, 128, DIM))
    bqk = np.concatenate([qkv_b[0:DIM] * scale, qkv_b[DIM:2 * DIM]])
    bqk_pp = np.ascontiguousarray(bqk.reshape(8, 128).T.astype(np.float32))
    bv = qkv_b[2 * DIM:3 * DIM].astype(np.float32)

    nb = x.shape[0]
    # per-window compacted key indices, padded to KC with slot 0
    idx_pad = np.zeros((nb, KC), dtype=np.int64)
    nk = np.zeros(nb, dtype=np.int64)
    for w in range(nb):
        iw = np.nonzero(mask[w])[0]
        nk[w] = len(iw)
        assert len(iw) <= KC
        idx_pad[w, 0:len(iw)] = iw

    x16 = x.astype(np.float16)
    xk16 = np.take_along_axis(x16, idx_pad[:, :, None], axis=1)  # [B,KC,512]
    slot = np.arange(KC)
    mbk_all = np.where(slot[None, :] < nk[:, None], EXP_SHIFT,
                       MASK_NEG).astype(np.float32)              # [B, KC]

    # exp(rpe bias) gathered per window in compacted key order:
    # ebw[w, kslot, h, q] = exp(B[q, idx_pad[w,kslot], h])
    eb = np.exp(rpe_table[rpe_index].reshape(N, N, H).astype(np.float32))
    eb16 = eb.astype(np.float16)                                 # [q, k, h]

    in_maps = []
    for core in range(n_cores):
        sl = slice(core * n_w, (core + 1) * n_w)
        # xt: [ci, 128, w*196+n] = x16[w, n, ci*128+c]
        xt = np.ascontiguousarray(
            x16[sl].reshape(n_w * N, 4, 128).transpose(1, 2, 0))
        xkt = np.ascontiguousarray(
            xk16[sl].reshape(n_w * KC, 4, 128).transpose(1, 2, 0))
        ebw = np.ascontiguousarray(
            eb16[:, idx_pad[sl], :]          # [q, n_w, KC, h]
            .transpose(1, 2, 3, 0)           # [n_w, KC, h, q]
            .reshape(n_w, KC, H * N))
        mbk = np.ascontiguousarray(mbk_all[sl].T)                # [KC, n_w]
        in_maps.append({
            "xt": xt, "xkt": xkt,
            "wqk": wqk_t, "wv": wv_t, "wp": wp_t,
            "bqk": bqk_pp, "bv": bv, "bp": proj_b.astype(np.float32),
            "ebw": ebw, "mbk": mbk,
        })
    return in_maps


def _host_prep(x, rpe_index, mask, qkv_w, qkv_b, proj_w, proj_b, rpe_table,
               n_w=W, n_cores=NCORES):
    """Shard + layout/dtype prep (numpy only). Returns per-core input maps."""
    x = np.asarray(x, dtype=np.float32)
    rpe_index = np.asarray(rpe_index).astype(np.int64)
    mask = np.asarray(mask).astype(np.int32)
    qkv_w = np.asarray(qkv_w, dtype=np.float32)
    qkv_b = np.asarray(qkv_b, dtype=np.float32)
    proj_w = np.asarray(proj_w, dtype=np.float32)
    proj_b = np.asarray(proj_b, dtype=np.float32)
    rpe_table = np.asarray(rpe_table, dtype=np.float32)

    scale = HD ** -0.5
    wq = qkv_w[0:DIM] * scale
    wk = qkv_w[DIM:2 * DIM]
    wv = qkv_w[2 * DIM:3 * DIM]
    wqk_t = np.concatenate([wq, wk], axis=0).T.astype(np.float16)  # [c, 1024]
    wv_t = wv.T.astype(np.float16)                                 # [c, 512]
    wp_t = proj_w.T.astype(np.float16)                             # [c, 512]
    wqk_t = np.ascontiguousarray(wqk_t.reshape(4, 128, 2 * DIM))
    wv_t = np.ascontiguousarray(wv_t.reshape(4, 128, DIM))
    wp_t = np.ascontiguousarray(wp_t.reshape(4, 128, DIM))

    bqk = np.concatenate([qkv_b[0:DIM] * scale, qkv_b[DIM:2 * DIM]])
    bqk_pp = np.ascontiguousarray(
        bqk.reshape(8, 128).T.astype(np.float32))                  # [128, 8]
    bv = qkv_b[2 * DIM:3 * DIM].astype(np.float32)

    tab = np.zeros((RPE, 128), dtype=np.float16)
    tab[:, 0:H] = rpe_table.astype(np.float16)

    # gather index stream: position j = cq*128 + p ; cq = c*196+q ; k = 98c+p
    cq = np.arange(2 * N)
    c = cq // N
    q = cq % N
    p = np.arange(128)
    k = (NKC * c)[:, None] + p[None, :]                            # [392, 128]
    valid = p[None, :] < NKC
    j_idx = np.where(valid, rpe_index[q[:, None] * N + np.minimum(k, N - 1)], 0)
    j_idx = j_idx.reshape(-1).astype(np.int16)
    n_per = _GQ * 128
    idx_w = np.zeros((16, (n_per // 16) * _NGATHER), dtype=np.int16)
    for gch in range(_NGATHER):
        blk = j_idx[gch * n_per:(gch + 1) * n_per].reshape(n_per // 16, 16).T
        idx_w[:, gch * (n_per // 16):(gch + 1) * (n_per // 16)] = blk
    idx_w = np.ascontiguousarray(np.tile(idx_w, (8, 1)))           # [128, .]

    ident = np.eye(128, dtype=np.float16)

    # v2: exp(rpe bias) in [k-part, chunk, head, q] layout (host-side prep)
    eb = np.exp(rpe_table[rpe_index].reshape(N, N, H).astype(np.float32))
    expbt = np.zeros((128, 2, H, N), dtype=np.float16)
    for cc in range(2):
        expbt[0:NKC, cc] = eb[:, cc * NKC:(cc + 1) * NKC, :].transpose(1, 2, 0)
    expbt = np.ascontiguousarray(expbt.reshape(128, 2 * H * N))

    in_maps = []
    for core in range(n_cores):
        xs = x[core * n_w:(core + 1) * n_w]
        ms = mask[core * n_w:(core + 1) * n_w]
        mbv = np.where(ms.astype(bool), EXP_SHIFT, MASK_NEG).astype(np.float32)
        mb = np.zeros((NKC, n_w * 2), dtype=np.float32)
        for wi in range(n_w):
            for cc in range(2):
                mb[:, 2 * wi + cc] = mbv[wi, cc * NKC:(cc + 1) * NKC]
        in_maps.append({
            "x": np.ascontiguousarray(xs),
            "wqk": wqk_t, "wv": wv_t, "wp": wp_t,
            "bqk": bqk_pp, "bv": bv, "bp": proj_b.astype(np.float32),
            "tab": tab, "idx": idx_w, "mb": np.ascontiguousarray(mb),
            "ident": ident, "expbt": expbt,
        })
    return in_maps


_NC_CACHE = {}
_VARIANT = "v4"


def kernel(x, rpe_index, mask, qkv_w, qkv_b, proj_w, proj_b, rpe_table,
           _trace=False):
    from concourse.bass_utils import run_bass_kernel_spmd
    use_v4 = (_VARIANT == "v4"
              and int(np.asarray(mask).sum(axis=1).max()) <= KC)
    if use_v4:
        in_maps = _host_prep_v4(x, rpe_index, mask, qkv_w, qkv_b, proj_w,
                                proj_b, rpe_table)
        if "nc4" not in _NC_CACHE:
            _NC_CACHE["nc4"] = _build_nc_v4(sgran="pair", anyeng=True,
                                            interleave=True)
        nc = _NC_CACHE["nc4"]
    else:
        in_maps = _host_prep(x, rpe_index, mask, qkv_w, qkv_b, proj_w,
                             proj_b, rpe_table)
        if "nc" not in _NC_CACHE:
            _NC_CACHE["nc"] = (_build_nc_v2(sgran="head")
                               if _VARIANT in ("v2", "v4")
                               else _build_nc(variant=_VARIANT))
        nc = _NC_CACHE["nc"]
    try:
        res = run_bass_kernel_spmd(nc, in_maps, core_ids=list(range(NCORES)),
                                   trace=_trace)
    except ModuleNotFoundError:
        # axon NTFF profiling hook unavailable in this container
        res = run_bass_kernel_spmd(nc, in_maps, core_ids=list(range(NCORES)),
                                   trace=False)
    kernel.last_results = res
    out = np.concatenate([r["out"] for r in res.results], axis=0)
    return out.reshape(B, N, DIM).astype(np.float32)

